# revision 10
# baseline (speedup 1.0000x reference)
"""Privacy-aware token pruning kernel for Trainium2 (8 NeuronCores, data parallel).

Host side replicates the reference's tiny (B,N) softmax pipeline with the same
eager jax ops (bit-identical to the oracle on this platform), then the Bass
kernel does the heavy work per core (4 batch rows each):
  - full descending sort of each row's 4096 softmax scores with index payload
    (max8 chunk sort + bitonic merge with a per-partition sign trick so every
    compare-exchange is uniform-descending)
  - index-ascending tie fixup to exactly match jax.lax.top_k ordering
  - indirect-DMA gather of the selected (top 2048) and pruned token vectors
  - pruned-token mean via PE matmul, broadcast add, contiguous writeback
"""

import numpy as np

B, N, D = 32, 4096, 256
K = N // 2
NCORES = 8
R = B // NCORES          # batch rows per core
P = 128
NCH = N // P             # 32 chunks per row
NEG = -3.0e38
S_MEAN = 0.05 / (2048.0 + 1e-10)   # MIXUP_ALPHA / remaining_count

_CACHE = {}


def _consts_np():
    c = np.arange(P) % NCH
    cols = [np.where(c % 2 == 0, 1.0, -1.0)]          # col 0: sigma_1
    for m in range(1, 6):                             # cols 1..5: sigma_m -> sigma_{m+1}
        sm = np.where(((c >> (m - 1)) & 1) == 0, 1.0, -1.0)
        sm1 = np.where(((c >> m) & 1) == 0, 1.0, -1.0)
        cols.append(sm * sm1)
    cols.append(np.ones(P))                           # col 6: ones (sum matmul lhsT)
    cols.append(np.zeros(P))                          # col 7: spare
    return np.stack(cols, axis=1).astype(np.float32)  # [128, 8]


def _build():
    import concourse.bacc as bacc
    import concourse.bass as bass
    import concourse.mybir as mybir
    import concourse.tile as tile

    f32, i32, u32, u8 = mybir.dt.float32, mybir.dt.int32, mybir.dt.uint32, mybir.dt.uint8
    i16 = mybir.dt.int16
    Alu = mybir.AluOpType

    nc = bacc.Bacc("TRN2", target_bir_lowering=False)

    soft_d = nc.dram_tensor("soft_c", [P, P], f32, kind="ExternalInput")
    seqf_d = nc.dram_tensor("seqf", [R * N, D], f32, kind="ExternalInput")
    consts_d = nc.dram_tensor("consts", [P, 8], f32, kind="ExternalInput")
    rows_d = nc.dram_tensor("rows_s", [1, P], f32, kind="ExternalInput")
    ident_d = nc.dram_tensor("ident", [P, P], f32, kind="ExternalInput")
    out_d = nc.dram_tensor("out", [R * K, D], f32, kind="ExternalOutput")

    with tile.TileContext(nc) as tc:
        with tc.tile_pool(name="main", bufs=1) as mp, \
             tc.tile_pool(name="i8p", bufs=2) as i8p, \
             tc.tile_pool(name="gath", bufs=1) as gp, \
             tc.tile_pool(name="ptr", bufs=2, space="PSUM") as ptr, \
             tc.tile_pool(name="psum", bufs=2, space="PSUM") as psp, \
             tc.tile_pool(name="pbc", bufs=2, space="PSUM") as pbc:

            consts = mp.tile([P, 8], f32, tag="consts")
            nc.sync.dma_start(consts[:], consts_d[:])
            rows_s = mp.tile([1, P], f32, tag="rows_s")
            nc.sync.dma_start(rows_s[:], rows_d[:])
            ident = mp.tile([P, P], f32, tag="ident")
            nc.sync.dma_start(ident[:], ident_d[:])
            zw = mp.tile([P, P], f32, tag="zw")       # working copy for max8 chain
            nc.sync.dma_start(zw[:], soft_d[:])

            iotaf = mp.tile([P, 1], f32, tag="iotaf")
            iotac = mp.tile([P, 1], i32, tag="iotac")
            nc.gpsimd.iota(iotac[:], pattern=[[1, 1]], base=0, channel_multiplier=128)
            nc.vector.tensor_copy(iotaf[:], iotac[:])

            # z = soft * sigma1 (per-partition scalar)
            nc.vector.tensor_scalar(zw[:], zw[:], consts[:, 0:1], None, op0=Alu.mult)

            V = mp.tile([P, P], f32, tag="V")         # sorted z values (layout A)
            V2 = mp.tile([P, P], f32, tag="V2")
            If = mp.tile([P, P], f32, tag="If")       # index payload (f32)
            If2 = mp.tile([P, P], f32, tag="If2")
            zB = mp.tile([P, P], f32, tag="zB")
            zB2 = mp.tile([P, P], f32, tag="zB2")
            iB = mp.tile([P, P], f32, tag="iB")
            iB2 = mp.tile([P, P], f32, tag="iB2")
            mk = mp.tile([P, P], u8, tag="mk")
            mk2 = mp.tile([P, 64], u8, tag="mk2")

            # ---- chunk sort: 16 x (max8 -> max_index -> match_replace) ----
            for k in range(16):
                sl = slice(8 * k, 8 * k + 8)
                nc.vector.max(V[:, sl], zw[:])
                i8 = i8p.tile([P, 8], u32, tag="i8")
                nc.vector.max_index(i8[:], V[:, sl], zw[:])
                nc.vector.match_replace(out=zw[:], in_to_replace=V[:, sl],
                                        in_values=zw[:], imm_value=NEG)
                nc.gpsimd.tensor_copy(If[:, sl], i8[:])   # uint32 -> f32 convert
            # global token id = chunk-local pos + p*128
            nc.vector.tensor_scalar(If[:], If[:], iotaf[:, 0:1], None, op0=Alu.add)

            def exchange(msk, za, zb, za_n, zb_n, ia, ib, ia_n, ib_n):
                """uniform descending compare-exchange (ping-pong form)."""
                nc.vector.tensor_tensor(msk, za, zb, op=Alu.is_ge)
                nc.vector.tensor_tensor(za_n, za, zb, op=Alu.max)
                nc.vector.tensor_tensor(zb_n, za, zb, op=Alu.min)
                nc.vector.tensor_copy(ia_n, ib)
                nc.vector.copy_predicated(ia_n, msk, ia)
                nc.vector.tensor_copy(ib_n, ia)
                nc.vector.copy_predicated(ib_n, msk, ib)

            def tr(dst, src, eng):
                pt = ptr.tile([P, P], f32, tag="tr")
                nc.tensor.transpose(pt[:], src[:], ident[:])
                if eng is nc.scalar:
                    nc.scalar.copy(dst[:], pt[:])
                else:
                    eng.tensor_copy(dst[:], pt[:])

            zc, zn, ic, inx = V, V2, If, If2
            for m in range(1, 6):
                # sigma_m -> sigma_{m+1} flip
                nc.vector.tensor_scalar(zc[:], zc[:], consts[:, m:m + 1], None,
                                        op0=Alu.mult)
                tr(zB, zc, nc.vector)
                tr(iB, ic, nc.scalar)
                bz, bz_n, bi, bi_n = zB, zB2, iB, iB2
                d = 1 << (m - 1)
                while d >= 1:
                    restricted = (m == 5 and d < 16)
                    ch = (8 // d) if restricted else (16 // d)

                    def full(t, d=d):
                        return t[:].rearrange("p (r ch two cl) -> p r ch two cl",
                                              r=R, ch=16 // d, two=2, cl=d)

                    def A_(t, d=d, ch=ch, restricted=restricted):
                        f = full(t, d)
                        return f[:, :, 0:ch, 0, :] if restricted else f[:, :, :, 0, :]

                    def B_(t, d=d, ch=ch, restricted=restricted):
                        f = full(t, d)
                        return f[:, :, 0:ch, 1, :] if restricted else f[:, :, :, 1, :]

                    msk = A_(mk)   # same strided structure as the operands
                    exchange(msk, A_(bz), B_(bz), A_(bz_n), B_(bz_n),
                             A_(bi), B_(bi), A_(bi_n), B_(bi_n))
                    if restricted:
                        def rest(t):
                            return t[:].rearrange("p (r c) -> p r c", r=R)[:, :, 16:32]
                        nc.scalar.copy(rest(bz_n), rest(bz))
                        nc.scalar.copy(rest(bi_n), rest(bi))
                    bz, bz_n = bz_n, bz
                    bi, bi_n = bi_n, bi
                    d >>= 1
                tr(zc, bz, nc.vector)
                tr(ic, bi, nc.scalar)
                for d in (64, 32, 16, 8, 4, 2, 1):
                    def a_ap(t, d=d):
                        return t[:].rearrange("p (jh two jl) -> p jh two jl",
                                              jh=64 // d, two=2, jl=d)[:, :, 0, :]

                    def b_ap(t, d=d):
                        return t[:].rearrange("p (jh two jl) -> p jh two jl",
                                              jh=64 // d, two=2, jl=d)[:, :, 1, :]

                    msk = a_ap(mk)   # same strided structure as the operands
                    exchange(msk, a_ap(zc), b_ap(zc), a_ap(zn), b_ap(zn),
                             a_ap(ic), b_ap(ic), a_ap(inx), b_ap(inx))
                    zc, zn = zn, zc
                    ic, inx = inx, ic

            # ---- tie fixup: one even+odd sweep on index payload ----
            for off in (0, 1):
                if off == 0:
                    def a_ap(t):
                        return t[:].rearrange("p (jh two) -> p jh two", two=2)[:, :, 0]

                    def b_ap(t):
                        return t[:].rearrange("p (jh two) -> p jh two", two=2)[:, :, 1]
                    w = 64
                else:
                    def a_ap(t):
                        return t[:, 1:127].rearrange("p (jh two) -> p jh two", two=2)[:, :, 0]

                    def b_ap(t):
                        return t[:, 1:127].rearrange("p (jh two) -> p jh two", two=2)[:, :, 1]
                    w = 63
                me = mk[:, 0:w]
                mg = mk2[:, 0:w]
                nc.vector.tensor_tensor(me, a_ap(zc), b_ap(zc), op=Alu.is_equal)
                nc.vector.tensor_tensor(mg, a_ap(ic), b_ap(ic), op=Alu.is_gt)
                nc.vector.tensor_tensor(me, me, mg, op=Alu.logical_and)
                tswap = zn[:, 0:w]                     # zn is scratch by now
                nc.vector.tensor_copy(tswap, a_ap(ic))
                nc.vector.copy_predicated(a_ap(ic), me, b_ap(ic))
                nc.vector.copy_predicated(b_ap(ic), me, tswap)

            # ---- build the dma_gather wrapped index tile ----
            # wrapped[q, F]: F < 512 selects rank u*16+q of row F//128 (u = F%128);
            # F >= 512 same for pruned ranks (c >= 16). idx value at sorted rank
            # c*128+j lives at ic[r*32+c, j]; with u = 8v+s: c = v (+16 for
            # pruned), j = s*16+q. Per s, a PE transpose of ic[:, s*16:(s+1)*16]
            # yields CsT[q, r*32+c]; strided copies then place sel/prn slices.
            wrapped = mp.tile([P, 2 * 512], i16, tag="wrapped")
            for s in range(8):
                cs = ptr.tile([16, P], f32, tag="cst")
                nc.tensor.transpose(cs[:], ic[:, s * 16:(s + 1) * 16], ident[:])
                csr = cs[:].rearrange("q (r c) -> q r c", r=R)
                for half, c0 in ((0, 0), (1, 16)):
                    dst = wrapped[0:16, half * 512:(half + 1) * 512].rearrange(
                        "q (r v s) -> q r v s", r=R, v=16, s=8)[:, :, :, s]
                    nc.vector.tensor_copy(dst, csr[:, :, c0:c0 + 16])
            # replicate the 16-partition wrap to all 8 q7-core partition groups
            for k in range(1, 8):
                nc.sync.dma_start(wrapped[16 * k:16 * (k + 1), :], wrapped[0:16, :])

            # ---- two big gathers: selected (rank-ordered) and pruned ----
            sel = gp.tile([P, 64 * D], f32, tag="sel")
            nc.gpsimd.dma_gather(
                out_ap=sel[:].rearrange("p (g d) -> p g d", g=64),
                in_ap=seqf_d[:],
                idxs_ap=wrapped[:, 0:512],
                num_idxs=R * K,
                num_idxs_reg=R * K,
                elem_size=D,
                single_packet=False,
            )
            prn = gp.tile([P, 64 * D], f32, tag="prn")
            nc.gpsimd.dma_gather(
                out_ap=prn[:].rearrange("p (g d) -> p g d", g=64),
                in_ap=seqf_d[:],
                idxs_ap=wrapped[:, 512:1024],
                num_idxs=R * K,
                num_idxs_reg=R * K,
                elem_size=D,
                single_packet=False,
            )

            # ---- per row: pruned sums, broadcast add, writeback ----
            # sel[p, r*16+gg, :] = token at rank gg*128+p of row r
            for r in range(R):
                psum = psp.tile([1, D], f32, tag="sum")
                for g in range(16):
                    nc.tensor.matmul(psum[:], lhsT=consts[:, 6:7],
                                     rhs=prn[:, (r * 16 + g) * D:(r * 16 + g + 1) * D],
                                     start=(g == 0), stop=(g == 15))
                mean_sb = mp.tile([1, D], f32, tag=f"mean{r}")
                nc.scalar.copy(mean_sb[:], psum[:])
                bc = pbc.tile([P, D], f32, tag="bc")
                nc.tensor.matmul(bc[:], lhsT=rows_s[:], rhs=mean_sb[:],
                                 start=True, stop=True)
                selr = sel[:, r * 16 * D:(r + 1) * 16 * D].rearrange(
                    "p (g d) -> p g d", g=16)
                nc.vector.tensor_tensor(
                    selr, selr,
                    bc[:].unsqueeze(1).broadcast_to([P, 16, D]),
                    op=Alu.add)
                nc.sync.dma_start(
                    out_d[r * K:(r + 1) * K, :].rearrange("(g t) d -> t g d", t=P),
                    selr)

    nc.compile()
    return nc


def _get_nc():
    if "nc" not in _CACHE:
        _CACHE["nc"] = _build()
    return _CACHE["nc"]


def _soft_host(attn_weights, uniform_noise):
    # replicate reference.py's ops exactly (eager jax, default backend)
    import jax
    import jax.numpy as jnp
    gumbel = -jnp.log(-jnp.log(jnp.asarray(uniform_noise) + 1e-10) + 1e-10)
    noisy = jnp.asarray(attn_weights) + 0.1 * gumbel
    soft = jax.nn.softmax(noisy / 0.5, axis=-1)
    return np.asarray(soft)


def make_in_maps(seq, attn_weights, uniform_noise):
    soft = _soft_host(attn_weights, uniform_noise)
    consts = _consts_np()
    rows_s = np.full((1, P), S_MEAN, dtype=np.float32)
    ident = np.eye(P, dtype=np.float32)
    in_maps = []
    for i in range(NCORES):
        rows = slice(R * i, R * (i + 1))
        in_maps.append({
            "soft_c": np.ascontiguousarray(soft[rows].reshape(R * NCH, P)),
            "seqf": np.ascontiguousarray(np.asarray(seq)[rows].reshape(R * N, D)),
            "consts": consts,
            "rows_s": rows_s,
            "ident": ident,
        })
    return in_maps


def kernel(seq, attn_weights, uniform_noise):
    from concourse.bass_utils import run_bass_kernel_spmd
    nc = _get_nc()
    in_maps = make_in_maps(seq, attn_weights, uniform_noise)
    res = run_bass_kernel_spmd(nc, in_maps, core_ids=list(range(NCORES)))
    out = np.concatenate([res.results[i]["out"] for i in range(NCORES)], axis=0)
    return out.reshape(B, K, D)


# revision 18
# speedup vs baseline: 1.3616x; 1.3616x over previous
"""Privacy-aware token pruning kernel for Trainium2 (8 NeuronCores, data parallel).

Host side replicates the reference's tiny (B,N) softmax pipeline with the same
eager jax ops (bit-identical to the oracle on this platform), then the Bass
kernel does the heavy work per core (4 batch rows each):
  - full descending sort of each row's 4096 softmax scores with index payload
    (max8 chunk sort + bitonic merge with a per-partition sign trick so every
    compare-exchange is uniform-descending)
  - index-ascending tie fixup to exactly match jax.lax.top_k ordering
  - indirect-DMA gather of the selected (top 2048) and pruned token vectors
  - pruned-token mean via PE matmul, broadcast add, contiguous writeback
"""

import numpy as np

B, N, D = 32, 4096, 256
K = N // 2
NCORES = 8
R = B // NCORES          # batch rows per core
P = 128
NCH = N // P             # 32 chunks per row
NEG = -3.0e38
S_MEAN = 0.05 / (2048.0 + 1e-10)   # MIXUP_ALPHA / remaining_count

_CACHE = {}


def _consts_np():
    c = np.arange(P) % NCH
    cols = [np.where(c % 2 == 0, 1.0, -1.0)]          # col 0: sigma_1
    for m in range(1, 6):                             # cols 1..5: sigma_m -> sigma_{m+1}
        sm = np.where(((c >> (m - 1)) & 1) == 0, 1.0, -1.0)
        sm1 = np.where(((c >> m) & 1) == 0, 1.0, -1.0)
        cols.append(sm * sm1)
    cols.append(np.ones(P))                           # col 6: ones (sum matmul lhsT)
    cols.append(np.arange(P) * 128.0)                 # col 7: p*128 (global id base)
    return np.stack(cols, axis=1).astype(np.float32)  # [128, 8]


def _build():
    import concourse.bacc as bacc
    import concourse.bass as bass
    import concourse.mybir as mybir
    import concourse.tile as tile

    f32, i32, u32, u8 = mybir.dt.float32, mybir.dt.int32, mybir.dt.uint32, mybir.dt.uint8
    i16 = mybir.dt.int16
    Alu = mybir.AluOpType

    nc = bacc.Bacc("TRN2", target_bir_lowering=False)

    soft_d = nc.dram_tensor("soft_c", [P, P], f32, kind="ExternalInput")
    seqf_d = nc.dram_tensor("seqf", [R * N, D], f32, kind="ExternalInput")
    consts_d = nc.dram_tensor("consts", [P, 8], f32, kind="ExternalInput")
    rows_d = nc.dram_tensor("rows_s", [1, P], f32, kind="ExternalInput")
    ident_d = nc.dram_tensor("ident", [P, P], f32, kind="ExternalInput")
    out_d = nc.dram_tensor("out", [R * K, D], f32, kind="ExternalOutput")

    with tile.TileContext(nc) as tc:
        with tc.tile_pool(name="main", bufs=1) as mp, \
             tc.tile_pool(name="i8p", bufs=2) as i8p, \
             tc.tile_pool(name="gath", bufs=2) as gp, \
             tc.tile_pool(name="ptr", bufs=2, space="PSUM") as ptr, \
             tc.tile_pool(name="psum", bufs=2, space="PSUM") as psp, \
             tc.tile_pool(name="pbc", bufs=2, space="PSUM") as pbc:

            consts = mp.tile([P, 8], f32, tag="consts")
            nc.sync.dma_start(consts[:], consts_d[:])
            rows_s = mp.tile([1, P], f32, tag="rows_s")
            nc.sync.dma_start(rows_s[:], rows_d[:])
            ident = mp.tile([P, P], f32, tag="ident")
            nc.sync.dma_start(ident[:], ident_d[:])
            zw = mp.tile([P, P], f32, tag="zw")       # working copy for max8 chain
            nc.sync.dma_start(zw[:], soft_d[:])

            # ---- independent of the sort: stream seq contiguously for sum_all.
            # token t = g*128 + p of each 2048-token block; PE ones-matmuls
            # accumulate per-row partial sums into psum_all[r, 0:512]
            # (two 256-wide partials per matmul, summed later).
            asums = []
            for r in range(R):
                ps = psp.tile([1, 512], f32, tag="sumA")
                for hb in range(2):
                    blk = 2 * r + hb
                    st = gp.tile([P, 16 * D], f32, tag="stream")
                    nc.sync.dma_start(
                        st[:].rearrange("p (g d) -> p g d", g=16),
                        seqf_d[blk * 2048:(blk + 1) * 2048, :].rearrange(
                            "(g p) d -> p g d", p=P))
                    for g2 in range(8):
                        nc.tensor.matmul(ps[:],
                                         lhsT=consts[:, 6:7],
                                         rhs=st[:, g2 * 512:(g2 + 1) * 512],
                                         start=(hb == 0 and g2 == 0),
                                         stop=(hb == 1 and g2 == 7))
                asum = mp.tile([1, 512], f32, tag=f"asumP{r}")
                nc.scalar.copy(asum[:], ps[:])
                asums.append(asum)

            # z = soft * sigma1 (per-partition scalar)
            nc.vector.tensor_scalar(zw[:], zw[:], consts[:, 0:1], None, op0=Alu.mult)

            V = mp.tile([P, P], f32, tag="V")         # sorted z values (layout A)
            V2 = mp.tile([P, P], f32, tag="V2")
            If = mp.tile([P, P], f32, tag="If")       # index payload (f32)
            If2 = mp.tile([P, P], f32, tag="If2")
            zB = mp.tile([P, P], f32, tag="zB")
            zB2 = mp.tile([P, P], f32, tag="zB2")
            iB = mp.tile([P, P], f32, tag="iB")
            iB2 = mp.tile([P, P], f32, tag="iB2")
            mk = mp.tile([P, P], u8, tag="mk")
            mk2 = mp.tile([P, 64], u8, tag="mk2")

            # ---- chunk sort: 16 x (max8 -> max_index -> match_replace) ----
            for k in range(16):
                sl = slice(8 * k, 8 * k + 8)
                nc.vector.max(V[:, sl], zw[:])
                i8 = i8p.tile([P, 8], u32, tag="i8")
                nc.vector.max_index(i8[:], V[:, sl], zw[:])
                nc.vector.match_replace(out=zw[:], in_to_replace=V[:, sl],
                                        in_values=zw[:], imm_value=NEG)
                nc.vector.tensor_copy(If[:, sl], i8[:])   # uint32 -> f32 convert
            # global token id = chunk-local pos + p*128 (consts col 7)
            nc.vector.tensor_scalar(If[:], If[:], consts[:, 7:8], None, op0=Alu.add)

            def exchange(msk, za, zb, za_n, zb_n, ia, ib, ia_n, ib_n):
                """uniform descending compare-exchange (ping-pong form)."""
                nc.vector.tensor_tensor(msk, za, zb, op=Alu.is_ge)
                nc.vector.tensor_tensor(za_n, za, zb, op=Alu.max)
                nc.vector.tensor_tensor(zb_n, za, zb, op=Alu.min)
                nc.vector.tensor_copy(ia_n, ib)
                nc.vector.copy_predicated(ia_n, msk, ia)
                nc.vector.tensor_copy(ib_n, ia)
                nc.vector.copy_predicated(ib_n, msk, ib)

            def tr(dst, src, eng):
                pt = ptr.tile([P, P], f32, tag="tr")
                nc.tensor.transpose(pt[:], src[:], ident[:])
                if eng is nc.scalar:
                    nc.scalar.copy(dst[:], pt[:])
                else:
                    eng.tensor_copy(dst[:], pt[:])

            zc, zn, ic, inx = V, V2, If, If2
            for m in range(1, 6):
                # sigma_m -> sigma_{m+1} flip
                nc.vector.tensor_scalar(zc[:], zc[:], consts[:, m:m + 1], None,
                                        op0=Alu.mult)
                tr(zB, zc, nc.vector)
                tr(iB, ic, nc.scalar)
                bz, bz_n, bi, bi_n = zB, zB2, iB, iB2
                d = 1 << (m - 1)
                while d >= 1:
                    restricted = (m == 5 and d < 16)
                    ch = (8 // d) if restricted else (16 // d)

                    def full(t, d=d):
                        return t[:].rearrange("p (r ch two cl) -> p r ch two cl",
                                              r=R, ch=16 // d, two=2, cl=d)

                    def A_(t, d=d, ch=ch, restricted=restricted):
                        f = full(t, d)
                        return f[:, :, 0:ch, 0, :] if restricted else f[:, :, :, 0, :]

                    def B_(t, d=d, ch=ch, restricted=restricted):
                        f = full(t, d)
                        return f[:, :, 0:ch, 1, :] if restricted else f[:, :, :, 1, :]

                    msk = A_(mk)   # same strided structure as the operands
                    exchange(msk, A_(bz), B_(bz), A_(bz_n), B_(bz_n),
                             A_(bi), B_(bi), A_(bi_n), B_(bi_n))
                    if restricted:
                        def rest(t):
                            return t[:].rearrange("p (r c) -> p r c", r=R)[:, :, 16:32]
                        nc.scalar.copy(rest(bz_n), rest(bz))
                        nc.scalar.copy(rest(bi_n), rest(bi))
                    bz, bz_n = bz_n, bz
                    bi, bi_n = bi_n, bi
                    d >>= 1
                tr(zc, bz, nc.vector)
                tr(ic, bi, nc.scalar)
                for d in (64, 32, 16, 8, 4, 2, 1):
                    def a_ap(t, d=d):
                        return t[:].rearrange("p (jh two jl) -> p jh two jl",
                                              jh=64 // d, two=2, jl=d)[:, :, 0, :]

                    def b_ap(t, d=d):
                        return t[:].rearrange("p (jh two jl) -> p jh two jl",
                                              jh=64 // d, two=2, jl=d)[:, :, 1, :]

                    msk = a_ap(mk)   # same strided structure as the operands
                    exchange(msk, a_ap(zc), b_ap(zc), a_ap(zn), b_ap(zn),
                             a_ap(ic), b_ap(ic), a_ap(inx), b_ap(inx))
                    zc, zn = zn, zc
                    ic, inx = inx, ic

            # ---- tie fixup: one even+odd sweep on index payload ----
            for off in (0, 1):
                if off == 0:
                    def a_ap(t):
                        return t[:].rearrange("p (jh two) -> p jh two", two=2)[:, :, 0]

                    def b_ap(t):
                        return t[:].rearrange("p (jh two) -> p jh two", two=2)[:, :, 1]
                    w = 64
                else:
                    def a_ap(t):
                        return t[:, 1:127].rearrange("p (jh two) -> p jh two", two=2)[:, :, 0]

                    def b_ap(t):
                        return t[:, 1:127].rearrange("p (jh two) -> p jh two", two=2)[:, :, 1]
                    w = 63
                me = mk[:, 0:w]
                mg = mk2[:, 0:w]
                nc.vector.tensor_tensor(me, a_ap(zc), b_ap(zc), op=Alu.is_equal)
                nc.vector.tensor_tensor(mg, a_ap(ic), b_ap(ic), op=Alu.is_gt)
                nc.vector.tensor_tensor(me, me, mg, op=Alu.logical_and)
                tswap = zn[:, 0:w]                     # zn is scratch by now
                nc.vector.tensor_copy(tswap, a_ap(ic))
                nc.vector.copy_predicated(a_ap(ic), me, b_ap(ic))
                nc.vector.copy_predicated(b_ap(ic), me, tswap)

            # ---- build the dma_gather wrapped index tile ----
            # wrapped[q, F]: F < 512 selects rank u*16+q of row F//128 (u = F%128);
            # F >= 512 same for pruned ranks (c >= 16). idx value at sorted rank
            # c*128+j lives at ic[r*32+c, j]; with u = 8v+s: c = v (+16 for
            # pruned), j = s*16+q. Per s, a PE transpose of ic[:, s*16:(s+1)*16]
            # yields CsT[q, r*32+c]; strided copies then place sel/prn slices.
            wrapped = mp.tile([P, 2 * 512], i16, tag="wrapped")
            for s in range(8):
                cs = ptr.tile([16, P], f32, tag="tr")
                nc.tensor.transpose(cs[:], ic[:, s * 16:(s + 1) * 16], ident[:])
                csr = cs[:].rearrange("q (r c) -> q r c", r=R)
                for half, c0 in ((0, 0), (1, 16)):
                    dst = wrapped[0:16, half * 512:(half + 1) * 512].rearrange(
                        "q (r v s) -> q r v s", r=R, v=16, s=8)[:, :, :, s]
                    nc.vector.tensor_copy(dst, csr[:, :, c0:c0 + 16])
            # replicate the 16-partition wrap to all 8 q7-core partition groups
            for k in range(1, 8):
                nc.sync.dma_start(wrapped[16 * k:16 * (k + 1), :], wrapped[0:16, :])

            # ---- gather selected tokens in two row-pair halves; per row:
            # sum_sel, mean = (sum_all - sum_sel)*s, broadcast add, writeback.
            # sel layout: sel[p, r2*16+gg, :] = token at rank gg*128+p of row r.
            for half in range(2):
                sel = gp.tile([P, 32 * D], f32, tag="sel")
                nc.gpsimd.dma_gather(
                    out_ap=sel[:].rearrange("p (g d) -> p g d", g=32),
                    in_ap=seqf_d[:],
                    idxs_ap=wrapped[:, half * 256:(half + 1) * 256],
                    num_idxs=2 * K,
                    num_idxs_reg=2 * K,
                    elem_size=D,
                    single_packet=False,
                )
                for r2 in range(2):
                    r = half * 2 + r2
                    psum_sel = psp.tile([1, 512], f32, tag="sumS")
                    for g2 in range(8):
                        nc.tensor.matmul(
                            psum_sel[:], lhsT=consts[:, 6:7],
                            rhs=sel[:, (r2 * 8 + g2) * 512:(r2 * 8 + g2 + 1) * 512],
                            start=(g2 == 0), stop=(g2 == 7))
                    # mean*alpha source = (sum_all - sum_sel), both as 2x256 partials
                    mean_sb = mp.tile([1, D], f32, tag=f"mean{r}")
                    acomb = mp.tile([1, D], f32, tag=f"acomb{r}")
                    nc.vector.tensor_tensor(
                        acomb[:], asums[r][:, 0:D], asums[r][:, D:2 * D],
                        op=Alu.add)
                    ssum = mp.tile([1, 512], f32, tag=f"ssum{r}")
                    nc.scalar.copy(ssum[:], psum_sel[:])
                    nc.vector.tensor_tensor(
                        mean_sb[:], ssum[:, 0:D], ssum[:, D:2 * D],
                        op=Alu.add)
                    nc.vector.tensor_tensor(
                        mean_sb[:], acomb[:], mean_sb[:], op=Alu.subtract)
                    bc = pbc.tile([P, D], f32, tag="bc")
                    nc.tensor.matmul(bc[:], lhsT=rows_s[:], rhs=mean_sb[:],
                                     start=True, stop=True)
                    selr = sel[:, r2 * 16 * D:(r2 + 1) * 16 * D].rearrange(
                        "p (g d) -> p g d", g=16)
                    nc.vector.tensor_tensor(
                        selr, selr,
                        bc[:].unsqueeze(1).broadcast_to([P, 16, D]),
                        op=Alu.add)
                    nc.sync.dma_start(
                        out_d[r * K:(r + 1) * K, :].rearrange("(g t) d -> t g d", t=P),
                        selr)

    nc.compile()
    return nc


def _get_nc():
    if "nc" not in _CACHE:
        _CACHE["nc"] = _build()
    return _CACHE["nc"]


def _soft_host(attn_weights, uniform_noise):
    # replicate reference.py's ops exactly (eager jax, default backend)
    import jax
    import jax.numpy as jnp
    gumbel = -jnp.log(-jnp.log(jnp.asarray(uniform_noise) + 1e-10) + 1e-10)
    noisy = jnp.asarray(attn_weights) + 0.1 * gumbel
    soft = jax.nn.softmax(noisy / 0.5, axis=-1)
    return np.asarray(soft)


def make_in_maps(seq, attn_weights, uniform_noise):
    soft = _soft_host(attn_weights, uniform_noise)
    consts = _consts_np()
    rows_s = np.full((1, P), S_MEAN, dtype=np.float32)
    ident = np.eye(P, dtype=np.float32)
    in_maps = []
    for i in range(NCORES):
        rows = slice(R * i, R * (i + 1))
        in_maps.append({
            "soft_c": np.ascontiguousarray(soft[rows].reshape(R * NCH, P)),
            "seqf": np.ascontiguousarray(np.asarray(seq)[rows].reshape(R * N, D)),
            "consts": consts,
            "rows_s": rows_s,
            "ident": ident,
        })
    return in_maps


def kernel(seq, attn_weights, uniform_noise):
    from concourse.bass_utils import run_bass_kernel_spmd
    nc = _get_nc()
    in_maps = make_in_maps(seq, attn_weights, uniform_noise)
    res = run_bass_kernel_spmd(nc, in_maps, core_ids=list(range(NCORES)))
    out = np.concatenate([res.results[i]["out"] for i in range(NCORES)], axis=0)
    return out.reshape(B, K, D)
